# revision 3
# baseline (speedup 1.0000x reference)
"""Trainium2 Bass kernel for nn_EquivariantGating.

Reference computation (after dead-code elimination of out1/out2):
    s : (z=512, d=256)   v : (z, 3)          [m1 = 1]
    out0[z,w] = pw0 * ( sum_{u,v} s[z,u] s[z,v] W1[u,v,w]
                        + INV_SQRT3 * |v_z|^2 * W4[w] )
    lin = out0 @ WL / sqrt(d_h)              -> (z, 2)
    return lin[:, :1], lin[:, 1:]  reshaped to (B, N, 1)

Because the final linear has only d_out=2 columns and everything between is
linear in W, the d_h=256 hidden axis can be folded away on the host:
    Weff[u,v,j] = scale * sum_w W1[u,v,w] WL[w,j]      (256, 256, 2)
    c[j]        = scale * INV_SQRT3 * sum_w W4[w] WL[w,j]
    lin[z,j]    = s_z^T Weff_j s_z + c[j] * |v_z|^2
The device evaluates the batched quadratic form, data-parallel over z
across 8 NeuronCores (64 nodes per core):
    t_j^T = Weff_j^T-contraction: t[v,z] = sum_u Weff_j[u,v] sT[u,z]  (PE)
    q_j   = t_j^T * sT  elementwise                                    (DVE)
    lin[j,z] = ones^T @ q_j  (partition reduction on PE)
               + cj^T @ (vT*vT)  (W4 term, K=3 matmul into same PSUM)
"""

import numpy as np

import concourse.bass as bass
import concourse.mybir as mybir
import concourse.tile as tile
from concourse.bass_utils import run_bass_kernel_spmd

F32 = mybir.dt.float32

N_CORES = 8
B, N = 2, 256
Z = B * N              # 512 flattened nodes
ZL = Z // N_CORES      # 64 nodes per core
D = 256                # scalar channels
INV_SQRT3 = 0.5773502691896258

_CACHE = {}


def build_nc():
    nc = bass.Bass()
    st = nc.declare_dram_parameter("st", [D, ZL], F32, isOutput=False)
    w = nc.declare_dram_parameter("w", [2, 2, 128, D], F32, isOutput=False)
    vt = nc.declare_dram_parameter("vt", [3, ZL], F32, isOutput=False)
    cj = nc.declare_dram_parameter("cj", [3, 2], F32, isOutput=False)
    out = nc.declare_dram_parameter("out", [1, 2 * ZL], F32, isOutput=True)

    with (
        nc.sbuf_tensor([128, 2 * ZL], F32) as ST,   # sT: chan b*128+p at col b*64+z
        nc.sbuf_tensor([128, 4 * D], F32) as W,     # Weff [u_p, (j, kb, v)]
        nc.sbuf_tensor([3, ZL], F32) as VT,
        nc.sbuf_tensor([3, 2], F32) as CJ,
        nc.sbuf_tensor([128, 1], F32) as ones,
        nc.sbuf_tensor([3, ZL], F32) as QV,
        nc.sbuf_tensor([128, 2 * ZL], F32) as Q0,
        nc.sbuf_tensor([128, 2 * ZL], F32) as Q1,
        nc.sbuf_tensor([1, 2 * ZL], F32) as OUTS,
        nc.psum_tensor([128, 2 * ZL], F32) as PT0,  # separate banks
        nc.psum_tensor([128, 2 * ZL], F32) as PT1,
        nc.psum_tensor([1, ZL], F32) as PO0,
        nc.psum_tensor([1, ZL], F32) as PO1,
        nc.semaphore("dma_in") as dma_in,
        nc.semaphore("vsem") as vsem,
        nc.semaphore("pesem") as pesem,
        nc.semaphore("dma_out") as dma_out,
        nc.Block() as block,
    ):
        Qs, PTs, POs = (Q0, Q1), (PT0, PT1), (PO0, PO1)

        @block.sync
        def _(sync):
            for b in range(2):
                sync.dma_start(out=ST[:, b * ZL:(b + 1) * ZL],
                               in_=st[b * 128:(b + 1) * 128, :]).then_inc(dma_in, 16)
            for j in range(2):
                for kb in range(2):
                    col = (j * 2 + kb) * D
                    sync.dma_start(out=W[:, col:col + D],
                                   in_=w[j, kb]).then_inc(dma_in, 16)
            sync.dma_start(out=VT[:, :], in_=vt[:, :]).then_inc(dma_in, 16)
            sync.dma_start(out=CJ[:, :], in_=cj[:, :]).then_inc(dma_in, 16)
            sync.wait_ge(vsem, 5)
            sync.dma_start(out=out[:, :], in_=OUTS[:, :]).then_inc(dma_out, 16)
            sync.wait_ge(dma_out, 16)

        @block.vector
        def _(vector):
            vector.memset(ones[:, :], 1.0)
            vector.wait_ge(dma_in, 128)
            vector.tensor_mul(QV[:, :], VT[:, :], VT[:, :]).then_inc(vsem, 1)
            for j in range(2):
                vector.wait_ge(pesem, j + 1)
                vector.tensor_mul(Qs[j][:, :], PTs[j][:, :],
                                  ST[:, :]).then_inc(vsem, 1)
            for j in range(2):
                vector.wait_ge(pesem, 3 + j)
                vector.tensor_copy(OUTS[0:1, j * ZL:(j + 1) * ZL],
                                   POs[j][:, :]).then_inc(vsem, 1)

        @block.tensor
        def _(tensor):
            tensor.wait_ge(dma_in, 128)
            for j in range(2):
                for mb in range(2):
                    for kb in range(2):
                        col = (j * 2 + kb) * D + mb * 128
                        mm = tensor.matmul(
                            PTs[j][:, mb * ZL:(mb + 1) * ZL],
                            W[:, col:col + 128],
                            ST[:, kb * ZL:(kb + 1) * ZL],
                            start=(kb == 0), stop=(kb == 1),
                        )
                mm.then_inc(pesem, 1)                      # pe = j+1
            for j in range(2):
                tensor.wait_ge(vsem, 2 + j)                # QV + Q_j ready
                tensor.matmul(POs[j][:, :], CJ[:, j:j + 1], QV[:, :],
                              start=True, stop=False)
                tensor.matmul(POs[j][:, :], ones[:, :], Qs[j][:, 0:ZL],
                              start=False, stop=False)
                tensor.matmul(POs[j][:, :], ones[:, :], Qs[j][:, ZL:2 * ZL],
                              start=False, stop=True).then_inc(pesem, 1)
    return nc


def _prepare(vectors, scalars, W1, W4, WL):
    d = scalars.shape[-1]
    d_h = W1.shape[-1]
    m1 = vectors.shape[-1] // 3
    pw0 = (1.0 / (d * d + m1 * m1)) ** 0.5
    scale = pw0 / np.sqrt(d_h)
    WL64 = WL.astype(np.float64)
    Weff = scale * (W1.astype(np.float64).reshape(d * d, d_h) @ WL64)
    # [j, u, v] -> [j, kb, p, v]
    wparam = np.ascontiguousarray(
        Weff.reshape(d, d, 2).transpose(2, 0, 1).reshape(2, 2, 128, d)
    ).astype(np.float32)
    c = (scale * INV_SQRT3) * (W4.astype(np.float64).reshape(d_h) @ WL64)
    cjarr = np.ascontiguousarray(
        np.tile(c.astype(np.float32)[None, :], (3, 1)))
    s = scalars.reshape(Z, d).astype(np.float32)
    v = vectors.reshape(Z, 3 * m1).astype(np.float32)
    in_maps = []
    for i in range(N_CORES):
        sl = slice(i * ZL, (i + 1) * ZL)
        in_maps.append({
            "st": np.ascontiguousarray(s[sl].T),
            "w": wparam,
            "vt": np.ascontiguousarray(v[sl].T),
            "cj": cjarr,
        })
    return in_maps


def kernel(vectors, scalars, W1, W2a, W2b, W3a, W3b, W4, WL):
    in_maps = _prepare(vectors, scalars, W1, W4, WL)
    if "nc" not in _CACHE:
        _CACHE["nc"] = build_nc()
    res = run_bass_kernel_spmd(_CACHE["nc"], in_maps, list(range(N_CORES)))
    lin = np.concatenate(
        [res.results[i]["out"].reshape(2, ZL) for i in range(N_CORES)],
        axis=1,
    ).T.astype(np.float32)  # (Z, 2)
    m_eqv = np.ascontiguousarray(lin[:, :1].reshape(B, N, 1))
    m_inv = np.ascontiguousarray(lin[:, 1:].reshape(B, N, 1))
    return (m_eqv, m_inv)


# revision 9
# speedup vs baseline: 1.0740x; 1.0740x over previous
"""Trainium2 Bass kernel for nn_EquivariantGating.

Reference computation (after dead-code elimination of out1/out2):
    s : (z=512, d=256)   v : (z, 3)          [m1 = 1]
    out0[z,w] = pw0 * ( sum_{u,v} s[z,u] s[z,v] W1[u,v,w]
                        + INV_SQRT3 * |v_z|^2 * W4[w] )
    lin = out0 @ WL / sqrt(d_h)              -> (z, 2)
    return lin[:, :1], lin[:, 1:]  reshaped to (B, N, 1)

Because the final linear has only d_out=2 columns and everything in between
is linear in the weights, the d_h=256 hidden axis folds away on the host:
    Weff[u,v,j] = scale * sum_w W1[u,v,w] WL[w,j]      (256, 256, 2)
    c[j]        = scale * INV_SQRT3 * sum_w W4[w] WL[w,j]
    lin[z,j]    = s_z^T Weff_j s_z + c[j] * |v_z|^2
The device evaluates the batched quadratic form, data-parallel over z
across 8 NeuronCores (64 nodes per core):
    PE : t_j[z,v] = sum_u sT[u,z] Weff_j[u,v]   (lhsT = sT stationary,
         rhs = Weff_j moving, PSUM accumulate over the two 128-row u blocks)
    DVE: lin[z,j] = reduce_add_v(s[z,v] * t_j[z,v])  seeded with the
         c_j*|v_z|^2 term via tensor_tensor_reduce's initial-value operand.
"""

import numpy as np

import concourse.bass as bass
import concourse.mybir as mybir
from concourse.bass_utils import run_bass_kernel_spmd

F32 = mybir.dt.float32
MULT = mybir.AluOpType.mult
ADD = mybir.AluOpType.add

N_CORES = 8
B, N = 2, 256
Z = B * N              # 512 flattened nodes
ZL = Z // N_CORES      # 64 nodes per core
D = 256                # scalar channels
INV_SQRT3 = 0.5773502691896258

_CACHE = {}


def build_nc():
    nc = bass.Bass()
    w = nc.declare_dram_parameter("w", [128, 4 * D], F32, isOutput=False)
    st = nc.declare_dram_parameter("st", [128, 2 * ZL], F32, isOutput=False)
    snv = nc.declare_dram_parameter("snv", [ZL, D + 5], F32, isOutput=False)
    out = nc.declare_dram_parameter("out", [ZL, 2], F32, isOutput=True)

    with (
        nc.sbuf_tensor([128, 4 * D], F32) as W,     # Weff [u_p, (j, kb, v)]
        nc.sbuf_tensor([128, 2 * ZL], F32) as ST,   # sT: chan kb*128+p at col kb*64+z
        nc.sbuf_tensor([ZL, D + 5], F32) as SNV,    # [s | v | c0 | c1]
        nc.sbuf_tensor([ZL, D], F32) as SCR0,       # ttr elementwise scratch
        nc.sbuf_tensor([ZL, D], F32) as SCR1,
        nc.sbuf_tensor([ZL, 3], F32) as SCRV0,
        nc.sbuf_tensor([ZL, 3], F32) as SCRV1,
        nc.sbuf_tensor([ZL, 2], F32) as TV,         # c_j*|v|^2 terms
        nc.sbuf_tensor([ZL, 2], F32) as R,          # sum_v s*t_j
        nc.sbuf_tensor([ZL, 2], F32) as OUTC,
        nc.psum_tensor([ZL, D], F32) as PT0,        # separate banks
        nc.psum_tensor([ZL, D], F32) as PT1,
        nc.semaphore("dma_in") as dma_in,
        nc.semaphore("dma_snv") as dma_snv,
        nc.semaphore("vsem") as vsem,
        nc.semaphore("tvsem") as tvsem,
        nc.semaphore("rsem") as rsem,
        nc.semaphore("pesem") as pesem,
        nc.semaphore("dma_out") as dma_out,
    ):
        with nc.Block() as block:
            PTs = (PT0, PT1)
            SCRs, SCRVs = (SCR0, SCR1), (SCRV0, SCRV1)
            S, V = SNV[:, 0:D], SNV[:, D:D + 3]

            @block.sync
            def _(sync):
                sync.dma_start(out=W[:, :], in_=w[:, :]).then_inc(dma_in, 16)
                sync.dma_start(out=ST[:, :], in_=st[:, :]).then_inc(dma_in, 16)
                sync.dma_start(out=SNV[:, :], in_=snv[:, :]).then_inc(dma_snv, 16)
                sync.wait_ge(vsem, 1)
                sync.dma_start(out=out[:, :], in_=OUTC[:, :]).then_inc(dma_out, 16)
                sync.wait_ge(dma_out, 16)

            @block.tensor
            def _(tensor):
                tensor.wait_ge(dma_in, 32)          # W + ST resident
                for j in range(2):
                    tensor.matmul(PTs[j][:, :], ST[:, 0:ZL],
                                  W[:, (2 * j) * D:(2 * j + 1) * D],
                                  start=True, stop=False)
                    tensor.matmul(PTs[j][:, :], ST[:, ZL:2 * ZL],
                                  W[:, (2 * j + 1) * D:(2 * j + 2) * D],
                                  start=False, stop=True).then_inc(pesem, 1)

            @block.vector
            def _(vector):
                vector.wait_ge(dma_snv, 16)         # SNV resident
                for j in range(2):
                    # TV[:, j] = sum_i (v_i * c_j) * v_i = c_j * |v|^2
                    vector.scalar_tensor_tensor(
                        out=SCRVs[j][:, :], in0=V,
                        scalar=SNV[:, D + 3 + j:D + 4 + j], in1=V,
                        op0=MULT, op1=MULT,
                        accum_out=TV[:, j:j + 1]).then_inc(tvsem, 1)
                for j in range(2):
                    vector.wait_ge(pesem, j + 1)
                    # R[:, j] = sum_v s[z,v] * t_j[z,v]
                    vector.scalar_tensor_tensor(
                        out=SCRs[j][:, :], in0=S, scalar=1.0,
                        in1=PTs[j][:, :], op0=MULT, op1=MULT,
                        accum_out=R[:, j:j + 1]).then_inc(rsem, 1)
                vector.wait_ge(tvsem, 2)
                vector.wait_ge(rsem, 2)
                vector.tensor_add(OUTC[:, :], R[:, :], TV[:, :]).then_inc(vsem, 1)
    return nc


def _prepare(vectors, scalars, W1, W4, WL):
    d = scalars.shape[-1]
    d_h = W1.shape[-1]
    m1 = vectors.shape[-1] // 3
    pw0 = (1.0 / (d * d + m1 * m1)) ** 0.5
    scale = pw0 / np.sqrt(d_h)
    WL64 = WL.astype(np.float64)
    Weff = scale * (W1.astype(np.float64).reshape(d * d, d_h) @ WL64)
    # [u, v, j] -> [p, (j, kb, v)] with u = kb*128 + p
    wparam = np.ascontiguousarray(
        Weff.reshape(d, d, 2).transpose(2, 0, 1)      # j, u, v
        .reshape(2, 2, 128, d)                        # j, kb, p, v
        .transpose(2, 0, 1, 3).reshape(128, 4 * d)    # p, (j kb v)
    ).astype(np.float32)
    c = (scale * INV_SQRT3) * (W4.astype(np.float64).reshape(d_h) @ WL64)
    s = scalars.reshape(Z, d).astype(np.float32)
    v = vectors.reshape(Z, 3 * m1).astype(np.float32)
    in_maps = []
    for i in range(N_CORES):
        sl = slice(i * ZL, (i + 1) * ZL)
        s_loc, v_loc = s[sl], v[sl]
        st = np.ascontiguousarray(
            s_loc.T.reshape(2, 128, ZL).transpose(1, 0, 2).reshape(128, 2 * ZL))
        ones = np.ones((ZL, 1), np.float64)
        snv = np.concatenate(
            [s_loc, v_loc, c[0] * ones, c[1] * ones], axis=1
        ).astype(np.float32)
        in_maps.append({"w": wparam, "st": st,
                        "snv": np.ascontiguousarray(snv)})
    return in_maps


def kernel(vectors, scalars, W1, W2a, W2b, W3a, W3b, W4, WL):
    in_maps = _prepare(vectors, scalars, W1, W4, WL)
    if "nc" not in _CACHE:
        _CACHE["nc"] = build_nc()
    res = run_bass_kernel_spmd(_CACHE["nc"], in_maps, list(range(N_CORES)))
    lin = np.concatenate([res.results[i]["out"] for i in range(N_CORES)],
                         axis=0).astype(np.float32)  # (Z, 2)
    m_eqv = np.ascontiguousarray(lin[:, :1].reshape(B, N, 1))
    m_inv = np.ascontiguousarray(lin[:, 1:].reshape(B, N, 1))
    return (m_eqv, m_inv)


# revision 10
# speedup vs baseline: 1.2124x; 1.1289x over previous
"""Trainium2 Bass kernel for nn_EquivariantGating.

Reference computation (after dead-code elimination of out1/out2):
    s : (z=512, d=256)   v : (z, 3)          [m1 = 1]
    out0[z,w] = pw0 * ( sum_{u,v} s[z,u] s[z,v] W1[u,v,w]
                        + INV_SQRT3 * |v_z|^2 * W4[w] )
    lin = out0 @ WL / sqrt(d_h)              -> (z, 2)
    return lin[:, :1], lin[:, 1:]  reshaped to (B, N, 1)

Because the final linear has only d_out=2 columns and everything in between
is linear in the weights, the d_h=256 hidden axis folds away on the host:
    Weff[u,v,j] = scale * sum_w W1[u,v,w] WL[w,j]      (256, 256, 2)
    c[j]        = scale * INV_SQRT3 * sum_w W4[w] WL[w,j]
    lin[z,j]    = s_z^T Weff_j s_z + c[j] * |v_z|^2
The device evaluates the batched quadratic form, data-parallel over z
across 8 NeuronCores (64 nodes per core):
    PE : t_j[z,v] = sum_u sT[u,z] Weff_j[u,v]   (lhsT = sT stationary,
         rhs = Weff_j moving, PSUM accumulate over the two 128-row u blocks)
    DVE: lin[z,j] = reduce_add_v(s[z,v] * t_j[z,v])  seeded with the
         c_j*|v_z|^2 term via tensor_tensor_reduce's initial-value operand.
"""

import numpy as np

import concourse.bass as bass
import concourse.mybir as mybir
from concourse.bass_utils import run_bass_kernel_spmd

F32 = mybir.dt.float32
MULT = mybir.AluOpType.mult
ADD = mybir.AluOpType.add

N_CORES = 8
B, N = 2, 256
Z = B * N              # 512 flattened nodes
ZL = Z // N_CORES      # 64 nodes per core
D = 256                # scalar channels
INV_SQRT3 = 0.5773502691896258

_CACHE = {}


def build_nc():
    nc = bass.Bass()
    w = nc.declare_dram_parameter("w", [128, 4 * D], F32, isOutput=False)
    st = nc.declare_dram_parameter("st", [128, 2 * ZL], F32, isOutput=False)
    snv = nc.declare_dram_parameter("snv", [ZL, D + 5], F32, isOutput=False)
    out = nc.declare_dram_parameter("out", [ZL, 2], F32, isOutput=True)

    with (
        nc.sbuf_tensor([128, 4 * D], F32) as W,     # Weff [u_p, (j, kb, v)]
        nc.sbuf_tensor([128, 2 * ZL], F32) as ST,   # sT: chan kb*128+p at col kb*64+z
        nc.sbuf_tensor([ZL, D + 5], F32) as SNV,    # [s | v | c0 | c1]
        nc.sbuf_tensor([ZL, D], F32) as SCR0,       # ttr elementwise scratch
        nc.sbuf_tensor([ZL, D], F32) as SCR1,
        nc.sbuf_tensor([ZL, 3], F32) as SCRV0,
        nc.sbuf_tensor([ZL, 3], F32) as SCRV1,
        nc.sbuf_tensor([ZL, 2], F32) as TV,         # c_j*|v|^2 terms
        nc.sbuf_tensor([ZL, 2], F32) as R,          # sum_v s*t_j
        nc.sbuf_tensor([ZL, 2], F32) as OUTC,
        nc.psum_tensor([ZL, 2 * D], F32) as PT,     # [z, (j, v)] one bank
        nc.semaphore("dma_in") as dma_in,
        nc.semaphore("dma_snv") as dma_snv,
        nc.semaphore("vsem") as vsem,
        nc.semaphore("tvsem") as tvsem,
        nc.semaphore("rsem") as rsem,
        nc.semaphore("pesem") as pesem,
        nc.semaphore("dma_out") as dma_out,
    ):
        with nc.Block() as block:
            SCRs, SCRVs = (SCR0, SCR1), (SCRV0, SCRV1)
            S, V = SNV[:, 0:D], SNV[:, D:D + 3]

            @block.sync
            def _(sync):
                sync.dma_start(out=W[:, :], in_=w[:, :]).then_inc(dma_in, 16)
                sync.dma_start(out=ST[:, :], in_=st[:, :]).then_inc(dma_in, 16)
                sync.dma_start(out=SNV[:, :], in_=snv[:, :]).then_inc(dma_snv, 16)
                sync.wait_ge(vsem, 1)
                # completion is guaranteed by the Block-exit dge drain; no
                # explicit wait on dma_out needed.
                sync.dma_start(out=out[:, :], in_=OUTC[:, :]).then_inc(dma_out, 16)

            @block.tensor
            def _(tensor):
                tensor.wait_ge(dma_in, 32)          # W + ST resident
                # t[z, (j,v)] accumulated over the two 128-row u blocks;
                # both j outputs fused into one N=512 moving operand.
                tensor.matmul(PT[:, :], ST[:, 0:ZL], W[:, 0:2 * D],
                              start=True, stop=False)
                tensor.matmul(PT[:, :], ST[:, ZL:2 * ZL], W[:, 2 * D:4 * D],
                              start=False, stop=True).then_inc(pesem, 1)

            @block.vector
            def _(vector):
                vector.wait_ge(dma_snv, 16)         # SNV resident
                for j in range(2):
                    # TV[:, j] = sum_i (v_i * c_j) * v_i = c_j * |v|^2
                    vector.scalar_tensor_tensor(
                        out=SCRVs[j][:, :], in0=V,
                        scalar=SNV[:, D + 3 + j:D + 4 + j], in1=V,
                        op0=MULT, op1=MULT,
                        accum_out=TV[:, j:j + 1]).then_inc(tvsem, 1)
                vector.wait_ge(pesem, 1)
                for j in range(2):
                    # R[:, j] = sum_v s[z,v] * t_j[z,v]
                    vector.scalar_tensor_tensor(
                        out=SCRs[j][:, :], in0=S, scalar=1.0,
                        in1=PT[:, j * D:(j + 1) * D], op0=MULT, op1=MULT,
                        accum_out=R[:, j:j + 1]).then_inc(rsem, 1)
                vector.wait_ge(tvsem, 2)
                vector.wait_ge(rsem, 2)
                vector.tensor_add(OUTC[:, :], R[:, :], TV[:, :]).then_inc(vsem, 1)
    return nc


def _prepare(vectors, scalars, W1, W4, WL):
    d = scalars.shape[-1]
    d_h = W1.shape[-1]
    m1 = vectors.shape[-1] // 3
    pw0 = (1.0 / (d * d + m1 * m1)) ** 0.5
    scale = pw0 / np.sqrt(d_h)
    WL64 = WL.astype(np.float64)
    Weff = scale * (W1.astype(np.float64).reshape(d * d, d_h) @ WL64)
    # [u, v, j] -> [p, (kb, j, v)] with u = kb*128 + p
    wparam = np.ascontiguousarray(
        Weff.reshape(2, 128, d, 2)                    # kb, p, v, j
        .transpose(1, 0, 3, 2).reshape(128, 4 * d)    # p, (kb j v)
    ).astype(np.float32)
    c = (scale * INV_SQRT3) * (W4.astype(np.float64).reshape(d_h) @ WL64)
    s = scalars.reshape(Z, d).astype(np.float32)
    v = vectors.reshape(Z, 3 * m1).astype(np.float32)
    in_maps = []
    for i in range(N_CORES):
        sl = slice(i * ZL, (i + 1) * ZL)
        s_loc, v_loc = s[sl], v[sl]
        st = np.ascontiguousarray(
            s_loc.T.reshape(2, 128, ZL).transpose(1, 0, 2).reshape(128, 2 * ZL))
        ones = np.ones((ZL, 1), np.float64)
        snv = np.concatenate(
            [s_loc, v_loc, c[0] * ones, c[1] * ones], axis=1
        ).astype(np.float32)
        in_maps.append({"w": wparam, "st": st,
                        "snv": np.ascontiguousarray(snv)})
    return in_maps


def kernel(vectors, scalars, W1, W2a, W2b, W3a, W3b, W4, WL):
    in_maps = _prepare(vectors, scalars, W1, W4, WL)
    if "nc" not in _CACHE:
        _CACHE["nc"] = build_nc()
    res = run_bass_kernel_spmd(_CACHE["nc"], in_maps, list(range(N_CORES)))
    lin = np.concatenate([res.results[i]["out"] for i in range(N_CORES)],
                         axis=0).astype(np.float32)  # (Z, 2)
    m_eqv = np.ascontiguousarray(lin[:, :1].reshape(B, N, 1))
    m_inv = np.ascontiguousarray(lin[:, 1:].reshape(B, N, 1))
    return (m_eqv, m_inv)
